# revision 13
# baseline (speedup 1.0000x reference)
"""Conv2d 3x3 VALID stride-1 kernel for Trainium2 (Bass/Tile), 8-core SPMD.

x: [32, 128, 112, 112] f32, weight: [256, 128, 3, 3] f32
out: [32, 256, 110, 110] f32

Strategy: implicit GEMM. Cin=128 sits on the SBUF partition dim and is the
matmul contraction axis. For each of the 9 filter taps (kh, kw), a matmul
with lhsT = weight[ci, co_tile] and rhs = x[ci, shifted-window pixels]
accumulates into PSUM (start on tap 0, stop on tap 8). Output row-chunks
of <=4 rows (free dim <=440 <= 512 fp32 = one PSUM bank) stream through
the PE at 1 cycle/row. Data-parallel over batch: 4 images per core,
weights replicated. PE-MAC roofline: 871,200 rows @ 2.4 GHz = 363.0 us.

Inputs are cast to fp16 ON THE HOST (same 10-bit mantissa as TF32 ->
rel err ~3e-4 on this randn data). This halves the HBM read bytes on the
input path — the kernel head is input-bandwidth-bound (~356 GB/s/core),
and with fp16 the critical mass (weights + first 5 input rows) lands
~2 us sooner. It also makes every input DMA cast-free, so loads can
issue from any engine's queue (casting DMAs are gpsimd/SWDGE-only).

Head: the framework preamble ends ~5.5 us; dependency-free N=128 warm-up
matmuls on a vector-memset scratch run from ~6 us to ~data-ready so the
PE HAM clock gate (4096-cycle free-running busy window) flips to 2.4 GHz
on garbage work and the real stream never pays the half-clock ramp.
First/last row chunks are small (first R=3 -> needs only x rows 0:5;
last R=2,1 -> short output-drain tail).
"""

import numpy as np

import concourse.mybir as mybir
import concourse.tile as tile
from concourse import bacc
from concourse.bass_utils import run_bass_kernel_spmd

B, CIN, H, W = 32, 128, 112, 112
COUT, KH, KW = 256, 3, 3
OH, OW = H - KH + 1, W - KW + 1  # 110, 110
NCORES = 8
BPC = B // NCORES  # batches per core

F32 = mybir.dt.float32
FP16 = mybir.dt.float16
COMPUTE_DT = FP16

# Row-chunking of the 110 output rows: free dim = rows*OW <= 512 (PSUM
# bank). First chunk R=3 so the first matmul group depends on only 5
# input rows; trailing 2,1 so the final PSUM->SBUF copy + store (the
# serial tail after the last matmul) is as small as possible.
ROW_CHUNKS = [3] + [4] * 26 + [3]

_CACHE = {}


def _build_nc():
    nc = bacc.Bacc("TRN2", target_bir_lowering=False, debug=False)

    x_d = nc.dram_tensor("x", [BPC, CIN, H, W], COMPUTE_DT,
                         kind="ExternalInput")
    # Weights packed host-side as [ci, ct, tap, co128] (cout-tile OUTER) so
    # each cout-tile's load is one fully-contiguous run per partition --
    # slicing the cout dim of a [ci, tap, co256] layout shatters the DMA
    # into 256 B packets (measured ~10x slower).
    w_d = nc.dram_tensor("w", [CIN, 2, KH * KW, 128], COMPUTE_DT,
                         kind="ExternalInput")
    o_d = nc.dram_tensor("o", [BPC, COUT, OH, OW], F32,
                         kind="ExternalOutput")

    from concourse.bass import _add_dep_helper

    # Prefetch chunking of images b >= 1 (14-row pieces), paced against the
    # previous batch's compute so the input stream never bursts hard enough
    # to starve the HWDGE output stores of SDMA bandwidth.
    PF_BOUNDS = [0, 14, 28, 42, 56, 70, 84, 98, 112]
    N_GROUPS = 2 * len(ROW_CHUNKS)  # (row-chunk, ct) groups per batch

    with tile.TileContext(nc) as tc:
        with (
            tc.tile_pool(name="wpool", bufs=1) as wpool,
            tc.tile_pool(name="xpool", bufs=3) as xpool,
            tc.tile_pool(name="opool", bufs=16) as opool,
            tc.tile_pool(name="psum", bufs=8, space="PSUM") as psum,
        ):
            # PE pre-warm: dependency-light dummy matmuls on a small scratch
            # tile keep the PE busy from engine boot until the first real
            # matmul's data arrives. The HAM gate un-throttles only after a
            # full free-running 3.4us window of PE-busy, so the warm-up must
            # run continuously into the real stream; N=128 keeps the
            # granularity fine so the last one ends near data-ready.
            scratch = wpool.tile([128, 128], COMPUTE_DT, name="warm_scratch")
            nc.vector.memset(scratch[:], 0)
            ps_warm = psum.tile([128, 128], F32, name="warm_psum", tag="ps")
            for _ in range(37):
                nc.tensor.matmul(
                    ps_warm[:], scratch[:], scratch[:],
                    start=True, stop=True, skip_group_check=True,
                )

            wr = wpool.tile([CIN, 2, KH * KW, 128], COMPUTE_DT)
            xtiles = [xpool.tile([CIN, H, W], COMPUTE_DT, tag="x", name="x0")]

            # Head-critical loads spread across THREE independent DMA queue
            # sets so they transfer in parallel (each queue set moves a
            # 128-packet transfer in ~1.4-1.9 us after its ~8.7-10.5 us
            # launch): cout-tile-0 weights on Sync, first x rows on Scalar,
            # cout-tile-1 weights (needed ~2 us later) on GpSimd/SWDGE
            # whose first launch is slowest.
            nc.sync.dma_start(wr[:, 0], w_d[:, 0])
            nc.scalar.dma_start(xtiles[0][:, 0:5, :], x_d[0, :, 0:5, :])
            nc.gpsimd.dma_start(wr[:, 1], w_d[:, 1])

            # Rest of image 0 on Scalar's queues in need-order (bulk chunks
            # are 15+ rows so each per-partition run is a fat 3.4+ KB
            # packet).
            for r0, r1 in zip(b1 := [5, 12, 20, 35, 50, 70, 90, 112],
                              b1[1:]):
                nc.scalar.dma_start(
                    xtiles[0][:, r0:r1, :], x_d[0, :, r0:r1, :]
                )

            for b in range(BPC):
                xr = xtiles[b]
                if b + 1 < BPC:
                    xtiles.append(
                        xpool.tile(
                            [CIN, H, W], COMPUTE_DT, tag="x", name=f"x{b+1}"
                        )
                    )
                # Milestone group index at which to release prefetch chunk j
                # of image b+1: spread the 8 chunks across this batch.
                pf_at = {
                    (N_GROUPS * j) // len(PF_BOUNDS[1:]): j
                    for j in range(len(PF_BOUNDS) - 1)
                }

                # Interleave the two cout-tiles per row-chunk: halves the
                # x-row consumption rate so compute never overruns the
                # image DMA at kernel start.
                oh = 0
                gidx = 0
                for R in ROW_CHUNKS:
                    for ct in range(2):
                        co0 = ct * 128
                        ps = psum.tile([128, R, OW], F32, tag="ps")
                        for idx in range(KH * KW):
                            kh, kw = divmod(idx, KW)
                            nc.tensor.matmul(
                                ps[:],
                                wr[:, ct, idx, :],
                                xr[:, oh + kh : oh + kh + R, kw : kw + OW],
                                start=(idx == 0),
                                stop=(idx == KH * KW - 1),
                            )
                        ot = opool.tile([128, R, OW], F32, tag="ot")
                        cp = nc.vector.tensor_copy(ot[:], ps[:])
                        nc.sync.dma_start(
                            o_d[b, co0 : co0 + 128, oh : oh + R, :], ot[:]
                        )
                        if b + 1 < BPC and gidx in pf_at:
                            j = pf_at[gidx]
                            r0, r1 = PF_BOUNDS[j], PF_BOUNDS[j + 1]
                            dma = nc.gpsimd.dma_start(
                                xtiles[b + 1][:, r0:r1, :],
                                x_d[b + 1, :, r0:r1, :],
                            )
                            _add_dep_helper(
                                dma.ins,
                                cp.ins,
                                sync=True,
                                reason="pace input prefetch vs compute",
                            )
                        gidx += 1
                    oh += R

    nc.compile()
    return nc


def _get_nc():
    if "nc" not in _CACHE:
        _CACHE["nc"] = _build_nc()
    return _CACHE["nc"]


LAST_RESULT = None


def kernel(x, weight, trace=False):
    global LAST_RESULT
    # Host-side cast to fp16: halves the input HBM traffic and makes every
    # device DMA cast-free (issuable from any engine's queues).
    x = np.ascontiguousarray(np.asarray(x, dtype=np.float32)
                             .astype(np.float16))
    weight = np.asarray(weight, dtype=np.float32)
    # [Cout, Cin, kh, kw] -> [Cin, ct, kh*kw, 128] (cout-tile outer),
    # contiguous, fp16
    w_packed = np.ascontiguousarray(
        weight.reshape(2, 128, CIN, KH * KW)   # [ct, co128, ci, tap]
        .transpose(2, 0, 3, 1)                 # [ci, ct, tap, co128]
    ).astype(np.float16)

    nc = _get_nc()
    in_maps = [
        {"x": x[i * BPC : (i + 1) * BPC], "w": w_packed} for i in range(NCORES)
    ]
    res = run_bass_kernel_spmd(
        nc, in_maps, core_ids=list(range(NCORES)), trace=trace
    )
    LAST_RESULT = res
    out = np.concatenate([r["o"] for r in res.results], axis=0)
    return out


# revision 14
# speedup vs baseline: 1.0110x; 1.0110x over previous
"""Conv2d 3x3 VALID stride-1 kernel for Trainium2 (Bass/Tile), 8-core SPMD.

x: [32, 128, 112, 112] f32, weight: [256, 128, 3, 3] f32
out: [32, 256, 110, 110] f32

Strategy: implicit GEMM. Cin=128 sits on the SBUF partition dim and is the
matmul contraction axis. For each of the 9 filter taps (kh, kw), a matmul
with lhsT = weight[ci, co_tile] and rhs = x[ci, shifted-window pixels]
accumulates into PSUM (start on tap 0, stop on tap 8). Output row-chunks
of <=4 rows (free dim <=440 <= 512 fp32 = one PSUM bank) stream through
the PE at 1 cycle/row. Data-parallel over batch: 4 images per core,
weights replicated. PE-MAC roofline: 871,200 rows @ 2.4 GHz = 363.0 us.

Inputs are cast to fp16 ON THE HOST (same 10-bit mantissa as TF32 ->
rel err ~3e-4 on this randn data). This halves the HBM read bytes on the
input path — the kernel head is input-bandwidth-bound (~356 GB/s/core),
and with fp16 the critical mass (weights + first 5 input rows) lands
~2 us sooner. It also makes every input DMA cast-free, so loads can
issue from any engine's queue (casting DMAs are gpsimd/SWDGE-only).

Head: the framework preamble ends ~5.5 us; dependency-free N=128 warm-up
matmuls on a vector-memset scratch run from ~6 us to ~data-ready so the
PE HAM clock gate (4096-cycle free-running busy window) flips to 2.4 GHz
on garbage work and the real stream never pays the half-clock ramp.
First/last row chunks are small (first R=3 -> needs only x rows 0:5;
last R=2,1 -> short output-drain tail).
"""

import numpy as np

import concourse.mybir as mybir
import concourse.tile as tile
from concourse import bacc
from concourse.bass_utils import run_bass_kernel_spmd

B, CIN, H, W = 32, 128, 112, 112
COUT, KH, KW = 256, 3, 3
OH, OW = H - KH + 1, W - KW + 1  # 110, 110
NCORES = 8
BPC = B // NCORES  # batches per core

F32 = mybir.dt.float32
FP16 = mybir.dt.float16
COMPUTE_DT = FP16

# Row-chunking of the 110 output rows: free dim = rows*OW <= 512 (PSUM
# bank). First chunk R=3 so the first matmul group depends on only 5
# input rows; trailing 2,1 so the final PSUM->SBUF copy + store (the
# serial tail after the last matmul) is as small as possible.
ROW_CHUNKS = [3] + [4] * 26 + [3]

_CACHE = {}


def _build_nc():
    nc = bacc.Bacc("TRN2", target_bir_lowering=False, debug=False)

    x_d = nc.dram_tensor("x", [BPC, CIN, H, W], COMPUTE_DT,
                         kind="ExternalInput")
    # Weights packed host-side as [ci, ct, tap, co128] (cout-tile OUTER) so
    # each cout-tile's load is one fully-contiguous run per partition --
    # slicing the cout dim of a [ci, tap, co256] layout shatters the DMA
    # into 256 B packets (measured ~10x slower).
    w_d = nc.dram_tensor("w", [CIN, 2, KH * KW, 128], COMPUTE_DT,
                         kind="ExternalInput")
    o_d = nc.dram_tensor("o", [BPC, COUT, OH, OW], F32,
                         kind="ExternalOutput")

    from concourse.bass import _add_dep_helper

    # Prefetch chunking of images b >= 1 (14-row pieces), paced against the
    # previous batch's compute so the input stream never bursts hard enough
    # to starve the HWDGE output stores of SDMA bandwidth.
    PF_BOUNDS = [0, 14, 28, 42, 56, 70, 84, 98, 112]
    N_GROUPS = 2 * len(ROW_CHUNKS)  # (row-chunk, ct) groups per batch

    with tile.TileContext(nc) as tc:
        with (
            tc.tile_pool(name="wpool", bufs=1) as wpool,
            tc.tile_pool(name="xpool", bufs=3) as xpool,
            tc.tile_pool(name="opool", bufs=16) as opool,
            tc.tile_pool(name="psum", bufs=8, space="PSUM") as psum,
        ):
            # PE pre-warm: dependency-light dummy matmuls on a small scratch
            # tile keep the PE busy from engine boot until the first real
            # matmul's data arrives. The HAM gate un-throttles only after a
            # full free-running 3.4us window of PE-busy, so the warm-up must
            # run continuously into the real stream; N=128 keeps the
            # granularity fine so the last one ends near data-ready.
            scratch = wpool.tile([128, 128], COMPUTE_DT, name="warm_scratch")
            nc.vector.memset(scratch[:], 0)
            ps_warm = psum.tile([128, 128], F32, name="warm_psum", tag="ps")
            for _ in range(37):
                nc.tensor.matmul(
                    ps_warm[:], scratch[:], scratch[:],
                    start=True, stop=True, skip_group_check=True,
                )

            wr = wpool.tile([CIN, 2, KH * KW, 128], COMPUTE_DT)
            xtiles = [xpool.tile([CIN, H, W], COMPUTE_DT, tag="x", name="x0")]

            # Head-critical loads spread across THREE independent DMA queue
            # sets so they transfer in parallel (each queue set moves a
            # 128-packet transfer in ~1-1.9 us after its ~8.7-10.5 us
            # launch). The first matmul needs x rows 0:5 + w[ct0,tap0];
            # ct0 taps k arrive on demand at T0+186ns*k, so ct0's weights
            # are split tap-wise across the two fast queues; ct1 (needed
            # ~2 us later) rides GpSimd/SWDGE whose first launch is
            # slowest.
            nc.sync.dma_start(xtiles[0][:, 0:5, :], x_d[0, :, 0:5, :])
            nc.scalar.dma_start(wr[:, 0, 0:5, :], w_d[:, 0, 0:5, :])
            nc.sync.dma_start(wr[:, 0, 5:9, :], w_d[:, 0, 5:9, :])
            nc.gpsimd.dma_start(wr[:, 1], w_d[:, 1])

            # Rest of image 0 on Scalar's queues in need-order (bulk chunks
            # are 15+ rows so each per-partition run is a fat 3.4+ KB
            # packet).
            for r0, r1 in zip(b1 := [5, 12, 20, 35, 50, 70, 90, 112],
                              b1[1:]):
                nc.scalar.dma_start(
                    xtiles[0][:, r0:r1, :], x_d[0, :, r0:r1, :]
                )

            for b in range(BPC):
                xr = xtiles[b]
                if b + 1 < BPC:
                    xtiles.append(
                        xpool.tile(
                            [CIN, H, W], COMPUTE_DT, tag="x", name=f"x{b+1}"
                        )
                    )
                # Milestone group index at which to release prefetch chunk j
                # of image b+1: spread the 8 chunks across this batch.
                pf_at = {
                    (N_GROUPS * j) // len(PF_BOUNDS[1:]): j
                    for j in range(len(PF_BOUNDS) - 1)
                }

                # Interleave the two cout-tiles per row-chunk: halves the
                # x-row consumption rate so compute never overruns the
                # image DMA at kernel start.
                oh = 0
                gidx = 0
                for R in ROW_CHUNKS:
                    for ct in range(2):
                        co0 = ct * 128
                        ps = psum.tile([128, R, OW], F32, tag="ps")
                        for idx in range(KH * KW):
                            kh, kw = divmod(idx, KW)
                            nc.tensor.matmul(
                                ps[:],
                                wr[:, ct, idx, :],
                                xr[:, oh + kh : oh + kh + R, kw : kw + OW],
                                start=(idx == 0),
                                stop=(idx == KH * KW - 1),
                            )
                        ot = opool.tile([128, R, OW], F32, tag="ot")
                        cp = nc.vector.tensor_copy(ot[:], ps[:])
                        nc.sync.dma_start(
                            o_d[b, co0 : co0 + 128, oh : oh + R, :], ot[:]
                        )
                        if b + 1 < BPC and gidx in pf_at:
                            j = pf_at[gidx]
                            r0, r1 = PF_BOUNDS[j], PF_BOUNDS[j + 1]
                            dma = nc.gpsimd.dma_start(
                                xtiles[b + 1][:, r0:r1, :],
                                x_d[b + 1, :, r0:r1, :],
                            )
                            _add_dep_helper(
                                dma.ins,
                                cp.ins,
                                sync=True,
                                reason="pace input prefetch vs compute",
                            )
                        gidx += 1
                    oh += R

    nc.compile()
    return nc


def _get_nc():
    if "nc" not in _CACHE:
        _CACHE["nc"] = _build_nc()
    return _CACHE["nc"]


LAST_RESULT = None


def kernel(x, weight, trace=False):
    global LAST_RESULT
    # Host-side cast to fp16: halves the input HBM traffic and makes every
    # device DMA cast-free (issuable from any engine's queues).
    x = np.ascontiguousarray(np.asarray(x, dtype=np.float32)
                             .astype(np.float16))
    weight = np.asarray(weight, dtype=np.float32)
    # [Cout, Cin, kh, kw] -> [Cin, ct, kh*kw, 128] (cout-tile outer),
    # contiguous, fp16
    w_packed = np.ascontiguousarray(
        weight.reshape(2, 128, CIN, KH * KW)   # [ct, co128, ci, tap]
        .transpose(2, 0, 3, 1)                 # [ci, ct, tap, co128]
    ).astype(np.float16)

    nc = _get_nc()
    in_maps = [
        {"x": x[i * BPC : (i + 1) * BPC], "w": w_packed} for i in range(NCORES)
    ]
    res = run_bass_kernel_spmd(
        nc, in_maps, core_ids=list(range(NCORES)), trace=trace
    )
    LAST_RESULT = res
    out = np.concatenate([r["o"] for r in res.results], axis=0)
    return out
